# revision 25
# baseline (speedup 1.0000x reference)
"""Trainium2 Bass kernel for the Calibrated Spectral Mixer (two-domain).

Math (per batch b):
    fx   = x @ W_in + b_in                      [N, H*DH]
    spec_h = inv_in_h^T @ fx_h                  [G, DH]   (contract over N)
    spec = LN_{(G,DH)}(spec) * ln_g + ln_b
    spec2_h = spec_h @ mlp_w
    out_h = inv_out_h @ spec2_h                 [N, DH]
    y    = concat_h(out_h) @ W_out + b_out      [N, C]

Device algebra (folded):
    E^T   = x^T-contraction:  E^T[c,g] = sum_n x[n,c] inv_in[n,g]
    specT_h[dh,g] = W_in_h^T @ E^T (+ b_in_h ⊗ s_in, s_in = sum_n inv_in[:,g])
    LN applied on packed specT, then spec2T_h = mlp_w^T @ affineT_h
    D[g,c] = sum_h spec2_h @ W_out_h    (heads summed when basis shared)
    yT[c,n] = D^T-contraction with inv_out^T  (+ b_out per-partition)

Sharding: data-parallel over B; core i computes batch i. No collectives.
If inv_in / inv_out are head-broadcast (they are, for the reference input
generator), only one head's basis is shipped/read (1 MB instead of 8 MB each).
"""

import sys

sys.path.insert(0, "/opt/trn_rl_repo")

import numpy as np

import concourse.bass as bass
import concourse.tile as tile
from concourse import bacc, mybir
from concourse.bass_utils import run_bass_kernel_spmd

B, N, C, H, DH, G = 8, 8192, 128, 8, 64, 32
HD = H * DH  # 512
NT = N // 128  # 64 n-tiles of 128 rows
LN_EPS = 1e-5
F32 = mybir.dt.float32

_cache: dict = {}


def _ind2t():
    a = np.zeros((2, 128), np.float32)
    a[0, 0:64] = 1.0
    a[1, 64:128] = 1.0
    return a


def _build(shared: bool):
    """Build + compile the per-core Bass program.

    shared=True: inv bases identical across heads (1-head basis shipped).
    shared=False: general per-head bases.
    """
    nc = bacc.Bacc("TRN2", target_bir_lowering=False, debug=False)

    x_d = nc.dram_tensor("x", [N, C], F32, kind="ExternalInput")
    w_in_d = nc.dram_tensor("w_in", [C, HD], F32, kind="ExternalInput")
    w_out_d = nc.dram_tensor("w_out", [HD, C], F32, kind="ExternalInput")
    mlp_d = nc.dram_tensor("mlp", [DH, DH], F32, kind="ExternalInput")
    lngt_d = nc.dram_tensor("lngt", [DH, H * G], F32, kind="ExternalInput")
    lnbt_d = nc.dram_tensor("lnbt", [DH, H * G], F32, kind="ExternalInput")
    b_in_d = nc.dram_tensor("b_in", [1, HD], F32, kind="ExternalInput")
    b_out_d = nc.dram_tensor("b_out", [C, 1], F32, kind="ExternalInput")
    if shared:
        invin_d = nc.dram_tensor("invin", [N, G], F32, kind="ExternalInput")
        invt_d = nc.dram_tensor("invt", [G, N], F32, kind="ExternalInput")
    else:
        invin_d = nc.dram_tensor("invin", [H, N, G], F32, kind="ExternalInput")
        invt_d = nc.dram_tensor("invt", [H * G, N], F32, kind="ExternalInput")
    yt_d = nc.dram_tensor("yT", [C, N], F32, kind="ExternalOutput")

    with tile.TileContext(nc) as tc:
        with (
            tc.tile_pool(name="res", bufs=1) as res,
            tc.tile_pool(name="work", bufs=1) as work,
            tc.tile_pool(name="yout", bufs=3) as yout,
            tc.tile_pool(name="ps", bufs=1, space="PSUM") as ps,
            tc.tile_pool(name="psy", bufs=2, space="PSUM") as psy,
        ):
            # ---- resident loads ----
            # x: 4 big tiles [128, 16*128]; tile i holds rows [2048*i, 2048*(i+1))
            xs = []
            for i in range(4):
                xt = res.tile([128, 16 * C], F32, tag=f"x{i}")
                nc.sync.dma_start(
                    out=xt[:].rearrange("p (t c) -> p t c", c=C),
                    in_=x_d[2048 * i : 2048 * (i + 1), :].rearrange(
                        "(t p) c -> p t c", p=128
                    ),
                )
                xs.append(xt)

            if shared:
                # [128, 64*32]: partition = n%128, free = (n//128, g)
                inv_sb = res.tile([128, NT * G], F32, tag="invin")
                nc.sync.dma_start(
                    out=inv_sb[:].rearrange("p (s g) -> p s g", g=G),
                    in_=invin_d[:].rearrange("(s p) g -> p s g", p=128),
                )
                # invT [G, N] -> [128, 4*2048]: partition = (n//2048)*32 + g
                invt_sb = res.tile([128, 4 * 2048], F32, tag="invt")
                for k in range(4):
                    nc.sync.dma_start(
                        out=invt_sb[k * G : (k + 1) * G, 0:2048],
                        in_=invt_d[:, k * 2048 : (k + 1) * 2048],
                    )
            else:
                # [128, H*64*32]: partition = n%128, free = (h, n//128, g)
                inv_sb = res.tile([128, H * NT * G], F32, tag="invin")
                nc.sync.dma_start(
                    out=inv_sb[:].rearrange("p (h s g) -> p h s g", g=G, h=H),
                    in_=invin_d[:].rearrange("h (s p) g -> p h s g", p=128),
                )
                # invT [H*G, N] -> [128, (kb, blk, f)]: partition = hg%128
                invt_sb = res.tile([128, 2 * 4 * 2048], F32, tag="invt")
                for kb in range(2):
                    for blk in range(4):
                        nc.sync.dma_start(
                            out=invt_sb[:, kb * 8192 + blk * 2048 :][:, 0:2048],
                            in_=invt_d[kb * 128 : (kb + 1) * 128,
                                       blk * 2048 : (blk + 1) * 2048],
                        )

            w_in_sb = res.tile([C, HD], F32, tag="w_in")
            nc.sync.dma_start(out=w_in_sb[:], in_=w_in_d[:])
            # W_out [512, 128] -> [64, (h, c)]: partition = dh, base 0
            w_out_sb = res.tile([DH, H * C], F32, tag="w_out")
            nc.sync.dma_start(
                out=w_out_sb[:].rearrange("p (h c) -> p h c", c=C),
                in_=w_out_d[:].rearrange("(h p) c -> p h c", p=DH),
            )
            mlp_sb = res.tile([DH, DH], F32, tag="mlp")
            nc.sync.dma_start(out=mlp_sb[:], in_=mlp_d[:])
            lngt_sb = res.tile([DH, H * G], F32, tag="lngt")
            nc.sync.dma_start(out=lngt_sb[:], in_=lngt_d[:])
            lnbt_sb = res.tile([DH, H * G], F32, tag="lnbt")
            nc.sync.dma_start(out=lnbt_sb[:], in_=lnbt_d[:])
            b_in_sb = res.tile([1, HD], F32, tag="b_in")
            nc.sync.dma_start(out=b_in_sb[:], in_=b_in_d[:])
            b_out_sb = res.tile([C, 1], F32, tag="b_out")
            nc.sync.dma_start(out=b_out_sb[:], in_=b_out_d[:])

            ones_sb = res.tile([128, 1], F32, tag="ones")
            nc.vector.memset(ones_sb[:], 1.0)
            ones_row = res.tile([1, DH], F32, tag="ones_row")
            nc.vector.memset(ones_row[:], 1.0)
            eps_sb = res.tile([1, 1], F32, tag="eps")
            nc.vector.memset(eps_sb[:], LN_EPS)

            # ---- phase A: E^T accumulation (and s_in row sums) ----
            ew = G if shared else H * G
            # es: E^T [128, ew] cols 0:ew ; s_in row [1, ew] cols ew:2*ew
            es = ps.tile([128, 2 * ew], F32, tag="es")
            for t in range(NT):
                lhs = xs[t // 16][:, (t % 16) * C : (t % 16 + 1) * C]
                if shared:
                    nc.tensor.matmul(
                        es[:, 0:G],
                        lhs,
                        inv_sb[:, t * G : (t + 1) * G],
                        start=(t == 0),
                        stop=(t == NT - 1),
                    )
                else:
                    for h in range(H):
                        nc.tensor.matmul(
                            es[:, h * G : (h + 1) * G],
                            lhs,
                            inv_sb[:, (h * NT + t) * G : (h * NT + t + 1) * G],
                            start=(t == 0),
                            stop=(t == NT - 1),
                        )
            # s_in: ones^T @ inv tiles -> [1, G] per head
            for t in range(NT):
                if shared:
                    nc.tensor.matmul(
                        es[0:1, ew : ew + G],
                        ones_sb[:],
                        inv_sb[:, t * G : (t + 1) * G],
                        start=(t == 0),
                        stop=(t == NT - 1),
                    )
                else:
                    for h in range(H):
                        nc.tensor.matmul(
                            es[0:1, ew + h * G : ew + (h + 1) * G],
                            ones_sb[:],
                            inv_sb[:, (h * NT + t) * G : (h * NT + t + 1) * G],
                            start=(t == 0),
                            stop=(t == NT - 1),
                        )

            e_sb = work.tile([128, ew], F32, tag="e_sb")
            nc.vector.tensor_copy(e_sb[:], es[:, 0:ew])
            s_sb = work.tile([1, ew], F32, tag="s_sb")
            nc.vector.tensor_copy(s_sb[:], es[0:1, ew : 2 * ew])

            # ---- phase B: specT packed [64, 256] ----
            # partition p = dh ; free f = h*32 + g (all PE ops at base 0)
            HG = H * G  # 256
            spec_ps = ps.tile([DH, HG], F32, tag="es")  # reuse es slot (freed)
            for h in range(H):
                reg = spec_ps[0:DH, h * G : (h + 1) * G]
                rhs = e_sb[:, 0:G] if shared else e_sb[:, h * G : (h + 1) * G]
                nc.tensor.matmul(
                    reg, w_in_sb[:, h * DH : (h + 1) * DH], rhs, start=True, stop=False
                )
                srhs = s_sb[0:1, 0:G] if shared else s_sb[0:1, h * G : (h + 1) * G]
                nc.tensor.matmul(
                    reg,
                    b_in_sb[0:1, h * DH : (h + 1) * DH],
                    srhs,
                    start=False,
                    stop=True,
                )

            spec_sb = work.tile([DH, HG], F32, tag="spec_sb")
            nc.vector.tensor_copy(spec_sb[:], spec_ps[:])

            # ---- phase C: LayerNorm over (G, DH) per head ----
            sq_sb = work.tile([DH, HG], F32, tag="sq_sb")
            nc.vector.tensor_mul(sq_sb[:], spec_sb[:], spec_sb[:])
            red1 = work.tile([DH, H], F32, tag="red1")
            nc.vector.reduce_sum(
                out=red1[:],
                in_=spec_sb[:].rearrange("p (h g) -> p h g", g=G),
                axis=mybir.AxisListType.X,
            )
            red2 = work.tile([DH, H], F32, tag="red2")
            nc.vector.reduce_sum(
                out=red2[:],
                in_=sq_sb[:].rearrange("p (h g) -> p h g", g=G),
                axis=mybir.AxisListType.X,
            )
            # st[0, 0:8] = per-head sums; st[0, 8:16] = per-head sumsq
            st = ps.tile([1, 2 * H], F32, tag="st")
            nc.tensor.matmul(st[0:1, 0:H], ones_sb[0:DH, 0:1], red1[:], start=True, stop=True)
            nc.tensor.matmul(st[0:1, H : 2 * H], ones_sb[0:DH, 0:1], red2[:], start=True, stop=True)

            mu_sb = work.tile([1, H], F32, tag="mu_sb")
            nc.scalar.activation(
                out=mu_sb[:],
                in_=st[0:1, 0:H],
                func=mybir.ActivationFunctionType.Copy,
                scale=1.0 / (G * DH),
            )
            msq_sb = work.tile([1, H], F32, tag="msq_sb")
            nc.scalar.activation(
                out=msq_sb[:],
                in_=st[0:1, H : 2 * H],
                func=mybir.ActivationFunctionType.Copy,
                scale=1.0 / (G * DH),
            )
            var_sb = work.tile([1, H], F32, tag="var_sb")
            nc.vector.tensor_mul(var_sb[:], mu_sb[:], mu_sb[:])
            nc.vector.tensor_sub(var_sb[:], msq_sb[:], var_sb[:])
            std_sb = work.tile([1, H], F32, tag="std_sb")
            nc.scalar.activation(
                out=std_sb[:],
                in_=var_sb[:],
                func=mybir.ActivationFunctionType.Sqrt,
                bias=eps_sb[:],
                scale=1.0,
            )
            rstd_sb = work.tile([1, H], F32, tag="rstd_sb")
            nc.vector.reciprocal(rstd_sb[:], std_sb[:])

            # broadcast [1, H] -> [1, 256] (repeat each head value 32x)
            mu256 = work.tile([1, HG], F32, tag="mu256")
            rstd256 = work.tile([1, HG], F32, tag="rstd256")
            for src, dst in ((mu_sb, mu256), (rstd_sb, rstd256)):
                ap = src[:, :]
                bc = bass.AP(tensor=ap.tensor, offset=ap.offset, ap=[ap.ap[0], ap.ap[1], [0, G]])
                nc.vector.tensor_copy(dst[:].rearrange("p (h g) -> p h g", g=G), bc)
            # broadcast across partitions via K=1 matmul
            mr = ps.tile([DH, 2 * HG], F32, tag="mr")
            nc.tensor.matmul(mr[:, 0:HG], ones_row[:], mu256[:], start=True, stop=True)
            nc.tensor.matmul(mr[:, HG : 2 * HG], ones_row[:], rstd256[:], start=True, stop=True)

            aff = work.tile([DH, HG], F32, tag="aff")
            nc.vector.tensor_sub(aff[:], spec_sb[:], mr[:, 0:HG])
            nc.vector.tensor_mul(aff[:], aff[:], mr[:, HG : 2 * HG])
            nc.vector.tensor_mul(aff[:], aff[:], lngt_sb[:])
            nc.vector.tensor_add(aff[:], aff[:], lnbt_sb[:])

            # ---- phase D: mlp + W_out fold ----
            dw = 2 * C
            # sp2d: spec2T packed [64, 0:256]; shared: D_sum accumulation [0:32, 256:384]
            sp2d = ps.tile([DH, HG + (C if shared else 0)], F32, tag="sp2d")
            dflat_ps = None if shared else ps.tile([G, H * C], F32, tag="dflat")
            for h in range(H):
                nc.tensor.matmul(
                    sp2d[0:DH, h * G : (h + 1) * G],
                    mlp_sb[:],
                    aff[0:DH, h * G : (h + 1) * G],
                    start=True,
                    stop=True,
                )
            spec2_sb = work.tile([DH, HG], F32, tag="spec2_sb")
            nc.vector.tensor_copy(spec2_sb[:], sp2d[:, 0:HG])
            for h in range(H):
                lhs = spec2_sb[0:DH, h * G : (h + 1) * G]
                rhs = w_out_sb[0:DH, h * C : (h + 1) * C]
                if shared:
                    # D_sum [G, C] accumulated over heads
                    nc.tensor.matmul(
                        sp2d[0:G, HG : HG + C],
                        lhs,
                        rhs,
                        start=(h == 0),
                        stop=(h == H - 1),
                    )
                else:
                    nc.tensor.matmul(
                        dflat_ps[0:G, h * C : (h + 1) * C],
                        lhs,
                        rhs,
                        start=True,
                        stop=True,
                    )
            if shared:
                # decode runs K=64 matmuls over invT partition halves (legal
                # bases are only {0, 32, 64}); build two zero-padded lhsT
                # variants: A = D on quarters 0/2, B = D on quarters 1/3.
                d_sb = work.tile([128, 2 * C], F32, tag="d_sb")
                nc.vector.memset(d_sb[:], 0.0)
                nc.vector.tensor_copy(d_sb[0:G, 0:C], sp2d[0:G, HG : HG + C])
                nc.vector.tensor_copy(d_sb[64 : 64 + G, 0:C], sp2d[0:G, HG : HG + C])
                nc.vector.tensor_copy(d_sb[G : 2 * G, C : 2 * C], sp2d[0:G, HG : HG + C])
                nc.vector.tensor_copy(d_sb[96:128, C : 2 * C], sp2d[0:G, HG : HG + C])
            else:
                # D_flat [(h*32+g)%128, (h//4)*128 + c] as two K=128 lhsT blocks
                d_sb = work.tile([128, dw], F32, tag="d_sb")
                for h in range(H):
                    nc.vector.tensor_copy(
                        d_sb[(h % 4) * G : (h % 4 + 1) * G,
                             (h // 4) * C : (h // 4 + 1) * C],
                        dflat_ps[0:G, h * C : (h + 1) * C],
                    )

            # ---- phase E: decode 16 chunks of 512 ----
            for c in range(16):
                yps = psy.tile([C, 512], F32, tag="y")
                if shared:
                    blk = c // 4
                    rhs = invt_sb[(blk // 2) * 64 : (blk // 2) * 64 + 64,
                                  (c % 4) * 512 : (c % 4 + 1) * 512]
                    lhs = d_sb[(blk // 2) * 64 : (blk // 2) * 64 + 64,
                               (blk % 2) * C : (blk % 2 + 1) * C]
                    nc.tensor.matmul(yps[:], lhs, rhs, start=True, stop=True)
                else:
                    for kb in range(2):
                        rhs = invt_sb[:, kb * 8192 + (c // 4) * 2048 + (c % 4) * 512 :][:, 0:512]
                        nc.tensor.matmul(
                            yps[:],
                            d_sb[:, kb * C : (kb + 1) * C],
                            rhs,
                            start=(kb == 0),
                            stop=(kb == 1),
                        )
                y_sb = yout.tile([C, 512], F32, tag="ysb")
                nc.vector.tensor_scalar_add(y_sb[:], yps[:], b_out_sb[:, 0:1])
                nc.sync.dma_start(out=yt_d[:, c * 512 : (c + 1) * 512], in_=y_sb[:])

    nc.compile()
    return nc


def kernel(x, W_in, b_in, mlp_w, ln_g, ln_b, W_out, b_out, inv_in, inv_out):
    x = np.ascontiguousarray(x, dtype=np.float32)
    inv_in = np.asarray(inv_in, dtype=np.float32)
    inv_out = np.asarray(inv_out, dtype=np.float32)

    shared = all(
        np.array_equal(a[0], a[h]) for a in (inv_in, inv_out) for h in range(1, H)
    )

    key = shared
    if key not in _cache:
        _cache[key] = _build(shared)
    nc = _cache[key]

    ln_gt = np.ascontiguousarray(np.asarray(ln_g, np.float32).T)  # [DH, G]
    lngt_p = np.tile(ln_gt, (1, H))  # [64, 256]
    ln_bt = np.ascontiguousarray(np.asarray(ln_b, np.float32).T)
    lnbt_p = np.tile(ln_bt, (1, H))

    if shared:
        invin_arr = np.ascontiguousarray(inv_in[0])  # [N, G]
        invt_arr = np.ascontiguousarray(inv_out[0].T)  # [G, N]
    else:
        invin_arr = inv_in
        invt_arr = np.ascontiguousarray(inv_out.transpose(0, 2, 1).reshape(H * G, N))

    common = {
        "w_in": np.ascontiguousarray(W_in, np.float32),
        "w_out": np.ascontiguousarray(W_out, np.float32),
        "mlp": np.ascontiguousarray(mlp_w, np.float32),
        "lngt": np.ascontiguousarray(lngt_p, np.float32),
        "lnbt": np.ascontiguousarray(lnbt_p, np.float32),
        "b_in": np.ascontiguousarray(np.asarray(b_in, np.float32).reshape(1, HD)),
        "b_out": np.ascontiguousarray(np.asarray(b_out, np.float32).reshape(C, 1)),
        "invin": invin_arr,
        "invt": invt_arr,
    }
    in_maps = [dict(common, x=np.ascontiguousarray(x[i])) for i in range(B)]

    res = run_bass_kernel_spmd(nc, in_maps, list(range(B)))
    out = np.empty((B, N, C), np.float32)
    for i in range(B):
        out[i] = res.results[i]["yT"].T
    return out


if __name__ == "__main__":
    rng = np.random.default_rng(0)
    ins = {
        "x": rng.standard_normal((B, N, C), np.float32),
        "W_in": rng.standard_normal((C, HD), np.float32) * 0.02,
        "b_in": np.zeros((HD,), np.float32),
        "mlp_w": rng.standard_normal((DH, DH), np.float32) * 0.02,
        "ln_g": np.ones((G, DH), np.float32),
        "ln_b": np.zeros((G, DH), np.float32),
        "W_out": rng.standard_normal((HD, C), np.float32) * 0.02,
        "b_out": np.zeros((C,), np.float32),
        "inv_in": np.broadcast_to(rng.standard_normal((1, N, G), np.float32), (H, N, G)).copy(),
        "inv_out": np.broadcast_to(rng.standard_normal((1, N, G), np.float32), (H, N, G)).copy(),
    }
    y = kernel(**ins)
    print("out", y.shape, y.dtype, np.abs(y).max())
